# revision 32
# baseline (speedup 1.0000x reference)
"""Trainium2 Bass kernel for BlockAxialDown (maxpool + axial attention + 1x1 conv + batchnorm).

Contract: kernel(**inputs) takes FULL unsharded inputs, returns FULL output.
Sharding: data-parallel over batch B=8 across 8 NeuronCores (1 image/core);
BatchNorm batch stats combined with a tiny (128,4) AllReduce; weights replicated.

v2: PE-based transposes (no Sync-engine DMA transposes), per-group batched
elementwise ops spread over Scalar/Vector/Pool engines, software-pipelined
issue order, maxpool interleaved with W-attention, single conv pass with the
relu'd conv output cached in SBUF (bf16) for the post-collective affine.
"""

import sys

import numpy as np

for _p in ("/opt/trn_rl_repo", "/root/.axon_site/_ro/trn_rl_repo"):
    if _p not in sys.path:
        sys.path.append(_p)

B, C, H, W = 8, 128, 256, 256
H2, W2 = 128, 128
E = 2 * C
NPOS = H2 * W2
NCORES = 8
BN_EPS = 1e-5
DH = C // 2
SCALE = DH ** -0.5

_CACHE = {}


def _build_program():
    import concourse.tile as tile
    from concourse import bacc, mybir, masks
    from concourse.alu_op_type import AluOpType
    from concourse.bass import broadcast_tensor_aps
    from contextlib import ExitStack

    F32 = mybir.dt.float32
    BF16 = mybir.dt.bfloat16
    AF = mybir.ActivationFunctionType
    AX = mybir.AxisListType
    P = 128

    nc = bacc.Bacc("TRN2", target_bir_lowering=False, debug=False, num_devices=NCORES)

    # ---- DRAM I/O ----
    x_d = nc.dram_tensor("x", [C, H, W], BF16, kind="ExternalInput").ap()
    wq_w_d = nc.dram_tensor("wq_w", [C, C], BF16, kind="ExternalInput").ap()
    wk_w_d = nc.dram_tensor("wk_w", [C, C], BF16, kind="ExternalInput").ap()
    wv_w_d = nc.dram_tensor("wv_w", [C, C], BF16, kind="ExternalInput").ap()
    wo_w_d = nc.dram_tensor("wo_w", [C, C], BF16, kind="ExternalInput").ap()
    wq_h_d = nc.dram_tensor("wq_h", [C, C], BF16, kind="ExternalInput").ap()
    wk_h_d = nc.dram_tensor("wk_h", [C, C], BF16, kind="ExternalInput").ap()
    wv_h_d = nc.dram_tensor("wv_h", [C, C], BF16, kind="ExternalInput").ap()
    wo_h_d = nc.dram_tensor("wo_h", [C, C], BF16, kind="ExternalInput").ap()
    bsum_d = nc.dram_tensor("bsum", [C, 1], F32, kind="ExternalInput").ap()
    convA_d = nc.dram_tensor("convA", [C, E], BF16, kind="ExternalInput").ap()
    convX_d = nc.dram_tensor("convX", [C, E], BF16, kind="ExternalInput").ap()
    gamma2_d = nc.dram_tensor("gamma2", [C, 2], F32, kind="ExternalInput").ap()
    beta2_d = nc.dram_tensor("beta2", [C, 2], F32, kind="ExternalInput").ap()
    out_d = nc.dram_tensor("out", [E, H2, W2], F32, kind="ExternalOutput").ap()
    stats_in_d = nc.dram_tensor("stats_in", [P, 4], F32).ap()
    stats_out_d = nc.dram_tensor("stats_out", [P, 4], F32, addr_space="Shared").ap()

    with tile.TileContext(nc) as tc, ExitStack() as ctx:
        const = ctx.enter_context(tc.tile_pool(name="const", bufs=1))
        cube = ctx.enter_context(tc.tile_pool(name="cube", bufs=1))
        stage = ctx.enter_context(tc.tile_pool(name="stage", bufs=3))
        work = ctx.enter_context(tc.tile_pool(name="work", bufs=2))
        stats = ctx.enter_context(tc.tile_pool(name="stats", bufs=1))
        psum = ctx.enter_context(tc.tile_pool(name="psum", bufs=1, space="PSUM"))

        # ---- constants ----
        def cload(name, ap_d, shape, dt):
            t = const.tile(shape, dt, name=name)
            nc.sync.dma_start(out=t[:], in_=ap_d)
            return t

        wq_w = cload("wq_w_t", wq_w_d, [C, C], BF16)
        wk_w = cload("wk_w_t", wk_w_d, [C, C], BF16)
        wv_w = cload("wv_w_t", wv_w_d, [C, C], BF16)
        wo_w = cload("wo_w_t", wo_w_d, [C, C], BF16)
        wq_h = cload("wq_h_t", wq_h_d, [C, C], BF16)
        wk_h = cload("wk_h_t", wk_h_d, [C, C], BF16)
        wv_h = cload("wv_h_t", wv_h_d, [C, C], BF16)
        wo_h = cload("wo_h_t", wo_h_d, [C, C], BF16)
        bsum = cload("bsum_t", bsum_d, [C, 1], F32)
        convA = cload("convA_t", convA_d, [C, E], BF16)
        convX = cload("convX_t", convX_d, [C, E], BF16)
        gamma2 = cload("gamma2_t", gamma2_d, [C, 2], F32)
        beta2 = cload("beta2_t", beta2_d, [C, 2], F32)
        ident = const.tile([P, P], BF16)
        masks.make_identity(nc, ident[:])

        xp = cube.tile([P, H2, W2], BF16)   # pooled input, channels on partitions
        acc = cube.tile([P, H2, W2], BF16)  # attention output accumulator
        ysb = [cube.tile([P, NPOS], BF16, name=f"ysb{i}") for i in range(2)]
        xp_f = xp[:].rearrange("c h w -> c (h w)")
        acc_f = acc[:].rearrange("c h w -> c (h w)")

        # ---- 2x2 maxpool of 8 input rows -> 4 pooled rows ----
        xv = x_d.rearrange("c (n r) w -> c n r w", r=8)

        def pool_iter(i):
            xin = stage.tile([P, 8, W], BF16, tag="xin")
            nc.sync.dma_start(out=xin[:], in_=xv[:, i])
            t = stage.tile([P, 8, W2], BF16, tag="wmax")
            xin4 = xin[:].rearrange("c r (w two) -> c r w two", two=2)
            nc.vector.tensor_max(t[:], xin4[:, :, :, 0], xin4[:, :, :, 1])
            t4 = t[:].rearrange("c (r2 two) w -> c r2 two w", two=2)
            nc.vector.tensor_max(xp[:, 4 * i:4 * i + 4, :], t4[:, :, 0, :], t4[:, :, 1, :])

        # ---- axial attention group (4 slices), 3-stage software pipeline ----
        def attn_A1(g, horiz):
            if horiz:
                wq, wk, wv = wq_h, wk_h, wv_h
                rhs_g = xp[:, :, 4 * g:4 * g + 4].rearrange("c h w -> c w h")
            else:
                wq, wk, wv = wq_w, wk_w, wv_w
                rhs_g = xp[:, 4 * g:4 * g + 4, :]
            # q/k/v into one fused 3-bank PSUM tile; qk evac on S, v on V
            qkv_ps = psum.tile([P, 1536], F32, tag="qkv", name="qkv_ps")
            nc.tensor.matmul(qkv_ps[:, 0:512], lhsT=wq[:], rhs=rhs_g, start=True, stop=True)
            nc.tensor.matmul(qkv_ps[:, 512:1024], lhsT=wk[:], rhs=rhs_g, start=True, stop=True)
            for s in range(4):
                nc.tensor.matmul(qkv_ps[:, 1024 + 128 * s:1152 + 128 * s],
                                 lhsT=rhs_g[:, s, :], rhs=wv[:], start=True, stop=True)
            qkv = work.tile([P, 1536], BF16, tag="qkv_sb", bufs=5)
            nc.scalar.copy(qkv[:, 0:1024], qkv_ps[:, 0:1024])
            nc.vector.tensor_copy(qkv[:, 1024:1536], qkv_ps[:, 1024:1536])
            return {"g": g, "horiz": horiz, "qkv": qkv}

        def attn_A2(st):
            qkv = st["qkv"]
            # dots for both heads in one 2-bank PSUM tile: cols 512*h + 128*s
            d_ps = psum.tile([P, 1024], F32, tag="d", name="d_ps")
            for s in range(4):
                for h in range(2):
                    hp = slice(64 * h, 64 * h + 64)
                    nc.tensor.matmul(d_ps[:, 512 * h + 128 * s:512 * h + 128 * s + 128],
                                     lhsT=qkv[hp, 128 * s:128 * s + 128],
                                     rhs=qkv[hp, 512 + 128 * s:640 + 128 * s],
                                     start=True, stop=True)
            e_sb = work.tile([P, 1024], BF16, tag="e", name="e_sb", bufs=3)
            nc.scalar.activation(e_sb[:], d_ps[:], AF.Exp, scale=SCALE)
            ev = e_sb[:].rearrange("c (hs j) -> c hs j", j=128)
            sums = work.tile([P, 8], F32, tag="sums", bufs=3)
            nc.vector.tensor_reduce(sums[:], ev, axis=AX.X, op=AluOpType.add)
            rcp = work.tile([P, 8], F32, tag="rcp", bufs=3)
            nc.vector.reciprocal(rcp[:], sums[:])
            rcp_v = rcp[:].rearrange("c (hs one) -> c hs one", one=1)
            _, rb = broadcast_tensor_aps(ev, rcp_v)
            nc.gpsimd.tensor_tensor(out=ev, in0=ev, in1=rb, op=AluOpType.mult)
            st["e_sb"] = e_sb
            return st

        # ---- stage B: transpose e, attend, project out, write acc ----
        def attn_B(st):
            g, horiz, e_sb, qkv = st["g"], st["horiz"], st["e_sb"], st["qkv"]
            wo = wo_h if horiz else wo_w
            et_ps = psum.tile([P, 1024], BF16, tag="et", name="et_ps")
            for s in range(4):
                for h in range(2):
                    o = 256 * s + 128 * h
                    nc.tensor.transpose(et_ps[:, o:o + 128],
                                        e_sb[:, 512 * h + 128 * s:512 * h + 128 * s + 128],
                                        ident[:])
            et = work.tile([P, 1024], BF16, tag="et_sb", bufs=3)
            nc.scalar.copy(et[:], et_ps[:])
            og_ps = psum.tile([P, 512], F32, tag="og", name="og_ps")
            for s in range(4):
                for h in range(2):
                    o = 256 * s + 128 * h
                    nc.tensor.matmul(og_ps[64 * h:64 * h + 64, 128 * s:128 * s + 128],
                                     lhsT=qkv[:, 1024 + 128 * s + 64 * h:1088 + 128 * s + 64 * h],
                                     rhs=et[:, o:o + 128], start=True, stop=True,
                                     tile_position=(0, 64 * h))
            og = work.tile([P, 512], BF16, tag="og_sb")
            nc.vector.tensor_copy(og[:], og_ps[:])
            yg_ps = psum.tile([P, 512], F32, tag="yg", name="yg_ps")
            nc.tensor.matmul(yg_ps[:], lhsT=wo[:], rhs=og[:], start=True, stop=True)
            if not horiz:
                # acc = yT_w + (bout_h + bout_w), contiguous write
                nc.scalar.activation(acc_f[:, 512 * g:512 * (g + 1)], yg_ps[:],
                                     AF.Identity, bias=bsum[:, 0:1], scale=1.0)
            else:
                # accumulate transposed: acc[:, h, w] += yg[:, (s=w, i=h)]
                acc_sl = acc[:, :, 4 * g:4 * g + 4]
                yg_r = yg_ps[:].rearrange("c (s i) -> c i s", s=4)
                nc.vector.tensor_add(acc_sl, acc_sl, yg_r)

        # ---- interleaved pool + attention, 3-stage software pipeline ----
        # lags: A1 at i, A2 at i-2, B at i-4; pool runs 3 steps ahead of A1 so
        # the projection matmuls never wait on fresh pool output. Within a step,
        # ready work (A2/B of older groups) issues before the new group's A1.
        seq = [(g, False) for g in range(H2 // 4)] + [(g, True) for g in range(W2 // 4)]
        n = len(seq)
        LAG_A2, LAG_B, POOL_AHEAD = 2, 4, 3
        for i in range(POOL_AHEAD):
            pool_iter(i)
        stash = {}
        for idx in range(n + LAG_B):
            j = idx - LAG_A2
            if 0 <= j < n:
                stash[j] = attn_A2(stash[j])
            j = idx - LAG_B
            if 0 <= j < n:
                attn_B(stash.pop(j))
            if idx + POOL_AHEAD < H2 // 4:
                pool_iter(idx + POOL_AHEAD)
            if idx < n:
                g, horiz = seq[idx]
                stash[idx] = attn_A1(g, horiz)

        # ---- relu over acc (split S/V) ----
        for j in range(4):
            sl = acc_f[:, 4096 * j:4096 * (j + 1)]
            if j % 2 == 0:
                nc.vector.tensor_scalar_max(sl, sl, 0.0)
            else:
                nc.scalar.activation(sl, sl, AF.Relu)

        # ---- conv pass (single): matmul + relu -> ysb (bf16), bn stats ----
        bnb = [stats.tile([P, 32, 6], F32, name=f"bnb{i}") for i in range(2)]
        for p in range(NPOS // 512):
            pos = slice(512 * p, 512 * (p + 1))
            for eh in range(2):
                ce = slice(128 * eh, 128 * eh + 128)
                yps = psum.tile([P, 512], F32, tag=("yg" if eh == 0 else "et"),
                                name=f"conv_ps{eh}")
                nc.tensor.matmul(yps[:], lhsT=convA[:, ce], rhs=acc_f[:, pos],
                                 start=True, stop=False)
                nc.tensor.matmul(yps[:], lhsT=convX[:, ce], rhs=xp_f[:, pos],
                                 start=False, stop=True)
                nc.scalar.activation(ysb[eh][:, pos], yps[:], AF.Relu)
                nc.vector.bn_stats(bnb[eh][:, p, :], ysb[eh][:, pos])

        # ---- global BN stats via AllReduce ----
        mv = stats.tile([P, 2, 2], F32)
        for eh in range(2):
            nc.vector.bn_aggr(mv[:, eh, :], bnb[eh][:])
        cc_in = stats.tile([P, 4], F32)
        for eh in range(2):
            # [mean, E[y^2]] per half; E[y^2] = var + mean^2
            nc.vector.tensor_copy(cc_in[:, 2 * eh:2 * eh + 1], mv[:, eh, 0:1])
            nc.vector.scalar_tensor_tensor(
                cc_in[:, 2 * eh + 1:2 * eh + 2],
                in0=mv[:, eh, 0:1], scalar=mv[:, eh, 0:1], in1=mv[:, eh, 1:2],
                op0=AluOpType.mult, op1=AluOpType.add)
        nc.sync.dma_start(out=stats_in_d, in_=cc_in[:])
        nc.gpsimd.collective_compute(
            "AllReduce", AluOpType.add,
            replica_groups=[list(range(NCORES))],
            ins=[stats_in_d], outs=[stats_out_d])
        gst = stats.tile([P, 4], F32)
        nc.sync.dma_start(out=gst[:], in_=stats_out_d)

        # ---- BN affine coefficients ----
        t0 = stats.tile([P, 4], F32)
        nc.vector.tensor_scalar_mul(t0[:], gst[:], 1.0 / NCORES)
        t0v = t0[:].rearrange("c (e two) -> c e two", two=2)
        m2 = stats.tile([P, 2], F32)
        veps = stats.tile([P, 2], F32)
        for eh in range(2):
            nc.vector.tensor_mul(m2[:, eh:eh + 1], t0v[:, eh, 0:1], t0v[:, eh, 0:1])
            nc.vector.scalar_tensor_tensor(
                veps[:, eh:eh + 1],
                in0=t0v[:, eh, 1:2], scalar=BN_EPS, in1=m2[:, eh:eh + 1],
                op0=AluOpType.add, op1=AluOpType.subtract)
        sd = stats.tile([P, 2], F32)
        nc.scalar.sqrt(sd[:], veps[:])
        rstd = stats.tile([P, 2], F32)
        nc.vector.reciprocal(rstd[:], sd[:])
        scl = stats.tile([P, 2], F32)
        nc.vector.tensor_mul(scl[:], gamma2[:], rstd[:])
        msc = stats.tile([P, 2], F32)
        means = stats.tile([P, 2], F32)
        nc.vector.tensor_copy(means[:, 0:1], t0v[:, 0, 0:1])
        nc.vector.tensor_copy(means[:, 1:2], t0v[:, 1, 0:1])
        nc.vector.tensor_mul(msc[:], means[:], scl[:])
        shift = stats.tile([P, 2], F32)
        nc.vector.tensor_sub(shift[:], beta2[:], msc[:])

        # ---- affine + store from cached ysb (Scalar: y*scl + shift) ----
        out_r = out_d.rearrange("(two c) h w -> two c (h w)", two=2)
        for p in range(NPOS // 512):
            pos = slice(512 * p, 512 * (p + 1))
            for eh in range(2):
                yo = work.tile([P, 512], F32, tag=f"yo{eh}", bufs=4)
                if eh == 0:
                    nc.scalar.activation(yo[:], ysb[0][:, pos], AF.Identity,
                                         scale=scl[:, 0:1], bias=shift[:, 0:1])
                else:
                    nc.vector.tensor_scalar(
                        yo[:], ysb[1][:, pos], scl[:, 1:2], shift[:, 1:2],
                        op0=AluOpType.mult, op1=AluOpType.add)
                nc.sync.dma_start(out=out_r[eh, :, pos], in_=yo[:])

    nc.finalize()
    return nc


def _get_program():
    if "nc" not in _CACHE:
        _CACHE["nc"] = _build_program()
    return _CACHE["nc"]


def _make_in_maps(x, Wq_h, Wkv_h, Wout_h, bout_h, Wq_w, Wkv_w, Wout_w, bout_w,
                  conv_w, gamma, beta):
    import ml_dtypes
    f = np.float32
    bf = ml_dtypes.bfloat16
    shared = {
        "wq_w": np.ascontiguousarray(np.asarray(Wq_w, f).astype(bf)),
        "wk_w": np.ascontiguousarray(np.asarray(Wkv_w, f)[:, :C].astype(bf)),
        "wv_w": np.ascontiguousarray(np.asarray(Wkv_w, f)[:, C:].astype(bf)),
        "wo_w": np.ascontiguousarray(np.asarray(Wout_w, f).astype(bf)),
        "wq_h": np.ascontiguousarray(np.asarray(Wq_h, f).astype(bf)),
        "wk_h": np.ascontiguousarray(np.asarray(Wkv_h, f)[:, :C].astype(bf)),
        "wv_h": np.ascontiguousarray(np.asarray(Wkv_h, f)[:, C:].astype(bf)),
        "wo_h": np.ascontiguousarray(np.asarray(Wout_h, f).astype(bf)),
        "bsum": np.ascontiguousarray((np.asarray(bout_h, f) + np.asarray(bout_w, f)).reshape(C, 1)),
        "convA": np.ascontiguousarray(np.asarray(conv_w, f)[:C, :].astype(bf)),
        "convX": np.ascontiguousarray(np.asarray(conv_w, f)[C:, :].astype(bf)),
        "gamma2": np.ascontiguousarray(np.asarray(gamma, f).reshape(2, C).T),
        "beta2": np.ascontiguousarray(np.asarray(beta, f).reshape(2, C).T),
    }
    xb = np.asarray(x, f).astype(bf)
    return [{**shared, "x": np.ascontiguousarray(xb[b])} for b in range(B)]


def run(trace=False, **inputs):
    from concourse.bass_utils import run_bass_kernel_spmd

    nc = _get_program()
    in_maps = _make_in_maps(**inputs)
    res = run_bass_kernel_spmd(nc, in_maps, list(range(NCORES)), trace=trace)
    out = np.stack([res.results[b]["out"] for b in range(B)], axis=0)
    return out, res


def kernel(**inputs):
    out, _ = run(trace=False, **inputs)
    return out


# revision 33
# speedup vs baseline: 1.0411x; 1.0411x over previous
"""Trainium2 Bass kernel for BlockAxialDown (maxpool + axial attention + 1x1 conv + batchnorm).

Contract: kernel(**inputs) takes FULL unsharded inputs, returns FULL output.
Sharding: data-parallel over batch B=8 across 8 NeuronCores (1 image/core);
BatchNorm batch stats combined with a tiny (128,4) AllReduce; weights replicated.

v2: PE-based transposes (no Sync-engine DMA transposes), per-group batched
elementwise ops spread over Scalar/Vector/Pool engines, software-pipelined
issue order, maxpool interleaved with W-attention, single conv pass with the
relu'd conv output cached in SBUF (bf16) for the post-collective affine.
"""

import sys

import numpy as np

for _p in ("/opt/trn_rl_repo", "/root/.axon_site/_ro/trn_rl_repo"):
    if _p not in sys.path:
        sys.path.append(_p)

B, C, H, W = 8, 128, 256, 256
H2, W2 = 128, 128
E = 2 * C
NPOS = H2 * W2
NCORES = 8
BN_EPS = 1e-5
DH = C // 2
SCALE = DH ** -0.5

_CACHE = {}


def _build_program():
    import concourse.tile as tile
    from concourse import bacc, mybir, masks
    from concourse.alu_op_type import AluOpType
    from concourse.bass import broadcast_tensor_aps
    from contextlib import ExitStack

    F32 = mybir.dt.float32
    BF16 = mybir.dt.bfloat16
    AF = mybir.ActivationFunctionType
    AX = mybir.AxisListType
    P = 128

    nc = bacc.Bacc("TRN2", target_bir_lowering=False, debug=False, num_devices=NCORES)

    # ---- DRAM I/O ----
    x_d = nc.dram_tensor("x", [C, H, W], BF16, kind="ExternalInput").ap()
    wq_w_d = nc.dram_tensor("wq_w", [C, C], BF16, kind="ExternalInput").ap()
    wk_w_d = nc.dram_tensor("wk_w", [C, C], BF16, kind="ExternalInput").ap()
    wv_w_d = nc.dram_tensor("wv_w", [C, C], BF16, kind="ExternalInput").ap()
    wo_w_d = nc.dram_tensor("wo_w", [C, C], BF16, kind="ExternalInput").ap()
    wq_h_d = nc.dram_tensor("wq_h", [C, C], BF16, kind="ExternalInput").ap()
    wk_h_d = nc.dram_tensor("wk_h", [C, C], BF16, kind="ExternalInput").ap()
    wv_h_d = nc.dram_tensor("wv_h", [C, C], BF16, kind="ExternalInput").ap()
    wo_h_d = nc.dram_tensor("wo_h", [C, C], BF16, kind="ExternalInput").ap()
    bsum_d = nc.dram_tensor("bsum", [C, 1], F32, kind="ExternalInput").ap()
    convA_d = nc.dram_tensor("convA", [C, E], BF16, kind="ExternalInput").ap()
    convX_d = nc.dram_tensor("convX", [C, E], BF16, kind="ExternalInput").ap()
    gamma2_d = nc.dram_tensor("gamma2", [C, 2], F32, kind="ExternalInput").ap()
    beta2_d = nc.dram_tensor("beta2", [C, 2], F32, kind="ExternalInput").ap()
    out_d = nc.dram_tensor("out", [E, H2, W2], F32, kind="ExternalOutput").ap()
    stats_in_d = nc.dram_tensor("stats_in", [P, 4], F32).ap()
    stats_out_d = nc.dram_tensor("stats_out", [P, 4], F32, addr_space="Shared").ap()

    with tile.TileContext(nc) as tc, ExitStack() as ctx:
        const = ctx.enter_context(tc.tile_pool(name="const", bufs=1))
        cube = ctx.enter_context(tc.tile_pool(name="cube", bufs=1))
        stage = ctx.enter_context(tc.tile_pool(name="stage", bufs=3))
        work = ctx.enter_context(tc.tile_pool(name="work", bufs=2))
        stats = ctx.enter_context(tc.tile_pool(name="stats", bufs=1))
        psum = ctx.enter_context(tc.tile_pool(name="psum", bufs=1, space="PSUM"))

        # ---- constants ----
        def cload(name, ap_d, shape, dt):
            t = const.tile(shape, dt, name=name)
            nc.sync.dma_start(out=t[:], in_=ap_d)
            return t

        wq_w = cload("wq_w_t", wq_w_d, [C, C], BF16)
        wk_w = cload("wk_w_t", wk_w_d, [C, C], BF16)
        wv_w = cload("wv_w_t", wv_w_d, [C, C], BF16)
        wo_w = cload("wo_w_t", wo_w_d, [C, C], BF16)
        wq_h = cload("wq_h_t", wq_h_d, [C, C], BF16)
        wk_h = cload("wk_h_t", wk_h_d, [C, C], BF16)
        wv_h = cload("wv_h_t", wv_h_d, [C, C], BF16)
        wo_h = cload("wo_h_t", wo_h_d, [C, C], BF16)
        bsum = cload("bsum_t", bsum_d, [C, 1], F32)
        convA = cload("convA_t", convA_d, [C, E], BF16)
        convX = cload("convX_t", convX_d, [C, E], BF16)
        gamma2 = cload("gamma2_t", gamma2_d, [C, 2], F32)
        beta2 = cload("beta2_t", beta2_d, [C, 2], F32)
        ident = const.tile([P, P], BF16)
        masks.make_identity(nc, ident[:])

        xp = cube.tile([P, H2, W2], BF16)   # pooled input, channels on partitions
        acc = cube.tile([P, H2, W2], BF16)  # attention output accumulator
        ysb = [cube.tile([P, NPOS], BF16, name=f"ysb{i}") for i in range(2)]
        xp_f = xp[:].rearrange("c h w -> c (h w)")
        acc_f = acc[:].rearrange("c h w -> c (h w)")

        # ---- 2x2 maxpool of 8 input rows -> 4 pooled rows ----
        xv = x_d.rearrange("c (n r) w -> c n r w", r=8)

        def pool_iter(i):
            xin = stage.tile([P, 8, W], BF16, tag="xin")
            nc.sync.dma_start(out=xin[:], in_=xv[:, i])
            t = stage.tile([P, 8, W2], BF16, tag="wmax")
            xin4 = xin[:].rearrange("c r (w two) -> c r w two", two=2)
            nc.vector.tensor_max(t[:], xin4[:, :, :, 0], xin4[:, :, :, 1])
            t4 = t[:].rearrange("c (r2 two) w -> c r2 two w", two=2)
            nc.vector.tensor_max(xp[:, 4 * i:4 * i + 4, :], t4[:, :, 0, :], t4[:, :, 1, :])

        # ---- axial attention group (4 slices), 3-stage software pipeline ----
        def attn_A1(g, horiz):
            if horiz:
                wq, wk, wv = wq_h, wk_h, wv_h
                rhs_g = xp[:, :, 4 * g:4 * g + 4].rearrange("c h w -> c w h")
            else:
                wq, wk, wv = wq_w, wk_w, wv_w
                rhs_g = xp[:, 4 * g:4 * g + 4, :]
            # q/k/v into one fused 3-bank PSUM tile; qk evac on S, v on V
            qkv_ps = psum.tile([P, 1536], F32, tag="qkv", name="qkv_ps")
            nc.tensor.matmul(qkv_ps[:, 0:512], lhsT=wq[:], rhs=rhs_g, start=True, stop=True)
            nc.tensor.matmul(qkv_ps[:, 512:1024], lhsT=wk[:], rhs=rhs_g, start=True, stop=True)
            for s in range(4):
                nc.tensor.matmul(qkv_ps[:, 1024 + 128 * s:1152 + 128 * s],
                                 lhsT=rhs_g[:, s, :], rhs=wv[:], start=True, stop=True)
            qkv = work.tile([P, 1536], BF16, tag="qkv_sb", bufs=5)
            nc.scalar.copy(qkv[:, 0:1024], qkv_ps[:, 0:1024])
            nc.vector.tensor_copy(qkv[:, 1024:1536], qkv_ps[:, 1024:1536])
            return {"g": g, "horiz": horiz, "qkv": qkv}

        def attn_A2(st):
            qkv = st["qkv"]
            # dots for both heads in one 2-bank PSUM tile: cols 512*h + 128*s
            d_ps = psum.tile([P, 1024], F32, tag="d", name="d_ps")
            for s in range(4):
                for h in range(2):
                    hp = slice(64 * h, 64 * h + 64)
                    nc.tensor.matmul(d_ps[:, 512 * h + 128 * s:512 * h + 128 * s + 128],
                                     lhsT=qkv[hp, 128 * s:128 * s + 128],
                                     rhs=qkv[hp, 512 + 128 * s:640 + 128 * s],
                                     start=True, stop=True)
            e_sb = work.tile([P, 1024], BF16, tag="e", name="e_sb", bufs=3)
            nc.scalar.activation(e_sb[:], d_ps[:], AF.Exp, scale=SCALE)
            ev = e_sb[:].rearrange("c (hs j) -> c hs j", j=128)
            sums = work.tile([P, 8], F32, tag="sums", bufs=3)
            nc.vector.tensor_reduce(sums[:], ev, axis=AX.X, op=AluOpType.add)
            rcp = work.tile([P, 8], F32, tag="rcp", bufs=3)
            nc.vector.reciprocal(rcp[:], sums[:])
            rcp_v = rcp[:].rearrange("c (hs one) -> c hs one", one=1)
            _, rb = broadcast_tensor_aps(ev, rcp_v)
            nc.gpsimd.tensor_tensor(out=ev, in0=ev, in1=rb, op=AluOpType.mult)
            st["e_sb"] = e_sb
            return st

        # ---- stage B: transpose e, attend, project out, write acc ----
        def attn_B(st):
            g, horiz, e_sb, qkv = st["g"], st["horiz"], st["e_sb"], st["qkv"]
            wo = wo_h if horiz else wo_w
            et_ps = psum.tile([P, 1024], BF16, tag="et", name="et_ps")
            for s in range(4):
                for h in range(2):
                    o = 256 * s + 128 * h
                    nc.tensor.transpose(et_ps[:, o:o + 128],
                                        e_sb[:, 512 * h + 128 * s:512 * h + 128 * s + 128],
                                        ident[:])
            et = work.tile([P, 1024], BF16, tag="et_sb")
            nc.scalar.copy(et[:], et_ps[:])
            og_ps = psum.tile([P, 512], F32, tag="og", name="og_ps")
            for s in range(4):
                for h in range(2):
                    o = 256 * s + 128 * h
                    nc.tensor.matmul(og_ps[64 * h:64 * h + 64, 128 * s:128 * s + 128],
                                     lhsT=qkv[:, 1024 + 128 * s + 64 * h:1088 + 128 * s + 64 * h],
                                     rhs=et[:, o:o + 128], start=True, stop=True,
                                     tile_position=(0, 64 * h))
            og = work.tile([P, 512], BF16, tag="og_sb")
            nc.vector.tensor_copy(og[:], og_ps[:])
            yg_ps = psum.tile([P, 512], F32, tag="yg", name="yg_ps")
            nc.tensor.matmul(yg_ps[:], lhsT=wo[:], rhs=og[:], start=True, stop=True)
            if not horiz:
                # acc = yT_w + (bout_h + bout_w), contiguous write
                nc.scalar.activation(acc_f[:, 512 * g:512 * (g + 1)], yg_ps[:],
                                     AF.Identity, bias=bsum[:, 0:1], scale=1.0)
            else:
                # accumulate transposed: acc[:, h, w] += yg[:, (s=w, i=h)]
                acc_sl = acc[:, :, 4 * g:4 * g + 4]
                yg_r = yg_ps[:].rearrange("c (s i) -> c i s", s=4)
                nc.vector.tensor_add(acc_sl, acc_sl, yg_r)

        # ---- interleaved pool + attention, 3-stage software pipeline ----
        # lags: A1 at i, A2 at i-2, B at i-4; pool runs 3 steps ahead of A1 so
        # the projection matmuls never wait on fresh pool output. Within a step,
        # ready work (A2/B of older groups) issues before the new group's A1.
        seq = [(g, False) for g in range(H2 // 4)] + [(g, True) for g in range(W2 // 4)]
        n = len(seq)
        LAG_A2, LAG_B, POOL_AHEAD = 2, 4, 3
        for i in range(POOL_AHEAD):
            pool_iter(i)
        stash = {}
        for idx in range(n + LAG_B):
            j = idx - LAG_A2
            if 0 <= j < n:
                stash[j] = attn_A2(stash[j])
            j = idx - LAG_B
            if 0 <= j < n:
                attn_B(stash.pop(j))
            if idx + POOL_AHEAD < H2 // 4:
                pool_iter(idx + POOL_AHEAD)
            if idx < n:
                g, horiz = seq[idx]
                stash[idx] = attn_A1(g, horiz)

        # ---- relu over acc (split S/V) ----
        for j in range(4):
            sl = acc_f[:, 4096 * j:4096 * (j + 1)]
            if j % 2 == 0:
                nc.vector.tensor_scalar_max(sl, sl, 0.0)
            else:
                nc.scalar.activation(sl, sl, AF.Relu)

        # ---- conv pass (single): matmul + relu -> ysb (bf16), bn stats ----
        bnb = [stats.tile([P, 32, 6], F32, name=f"bnb{i}") for i in range(2)]
        for p in range(NPOS // 512):
            pos = slice(512 * p, 512 * (p + 1))
            for eh in range(2):
                ce = slice(128 * eh, 128 * eh + 128)
                yps = psum.tile([P, 512], F32, tag=("yg" if eh == 0 else "et"),
                                name=f"conv_ps{eh}")
                nc.tensor.matmul(yps[:], lhsT=convA[:, ce], rhs=acc_f[:, pos],
                                 start=True, stop=False)
                nc.tensor.matmul(yps[:], lhsT=convX[:, ce], rhs=xp_f[:, pos],
                                 start=False, stop=True)
                nc.scalar.activation(ysb[eh][:, pos], yps[:], AF.Relu)
                nc.vector.bn_stats(bnb[eh][:, p, :], ysb[eh][:, pos])

        # ---- global BN stats via AllReduce ----
        mv = stats.tile([P, 2, 2], F32)
        for eh in range(2):
            nc.vector.bn_aggr(mv[:, eh, :], bnb[eh][:])
        cc_in = stats.tile([P, 4], F32)
        for eh in range(2):
            # [mean, E[y^2]] per half; E[y^2] = var + mean^2
            nc.vector.tensor_copy(cc_in[:, 2 * eh:2 * eh + 1], mv[:, eh, 0:1])
            nc.vector.scalar_tensor_tensor(
                cc_in[:, 2 * eh + 1:2 * eh + 2],
                in0=mv[:, eh, 0:1], scalar=mv[:, eh, 0:1], in1=mv[:, eh, 1:2],
                op0=AluOpType.mult, op1=AluOpType.add)
        nc.sync.dma_start(out=stats_in_d, in_=cc_in[:])
        nc.gpsimd.collective_compute(
            "AllReduce", AluOpType.add,
            replica_groups=[list(range(NCORES))],
            ins=[stats_in_d], outs=[stats_out_d])
        gst = stats.tile([P, 4], F32)
        nc.sync.dma_start(out=gst[:], in_=stats_out_d)

        # ---- BN affine coefficients ----
        t0 = stats.tile([P, 4], F32)
        nc.vector.tensor_scalar_mul(t0[:], gst[:], 1.0 / NCORES)
        t0v = t0[:].rearrange("c (e two) -> c e two", two=2)
        m2 = stats.tile([P, 2], F32)
        veps = stats.tile([P, 2], F32)
        for eh in range(2):
            nc.vector.tensor_mul(m2[:, eh:eh + 1], t0v[:, eh, 0:1], t0v[:, eh, 0:1])
            nc.vector.scalar_tensor_tensor(
                veps[:, eh:eh + 1],
                in0=t0v[:, eh, 1:2], scalar=BN_EPS, in1=m2[:, eh:eh + 1],
                op0=AluOpType.add, op1=AluOpType.subtract)
        sd = stats.tile([P, 2], F32)
        nc.scalar.sqrt(sd[:], veps[:])
        rstd = stats.tile([P, 2], F32)
        nc.vector.reciprocal(rstd[:], sd[:])
        scl = stats.tile([P, 2], F32)
        nc.vector.tensor_mul(scl[:], gamma2[:], rstd[:])
        msc = stats.tile([P, 2], F32)
        means = stats.tile([P, 2], F32)
        nc.vector.tensor_copy(means[:, 0:1], t0v[:, 0, 0:1])
        nc.vector.tensor_copy(means[:, 1:2], t0v[:, 1, 0:1])
        nc.vector.tensor_mul(msc[:], means[:], scl[:])
        shift = stats.tile([P, 2], F32)
        nc.vector.tensor_sub(shift[:], beta2[:], msc[:])

        # ---- affine + store from cached ysb (Scalar: y*scl + shift) ----
        out_r = out_d.rearrange("(two c) h w -> two c (h w)", two=2)
        for p in range(NPOS // 512):
            pos = slice(512 * p, 512 * (p + 1))
            for eh in range(2):
                yo = work.tile([P, 512], F32, tag=f"yo{eh}")
                if eh == 0:
                    nc.scalar.activation(yo[:], ysb[0][:, pos], AF.Identity,
                                         scale=scl[:, 0:1], bias=shift[:, 0:1])
                else:
                    nc.vector.tensor_scalar(
                        yo[:], ysb[1][:, pos], scl[:, 1:2], shift[:, 1:2],
                        op0=AluOpType.mult, op1=AluOpType.add)
                nc.sync.dma_start(out=out_r[eh, :, pos], in_=yo[:])

    nc.finalize()
    return nc


def _get_program():
    if "nc" not in _CACHE:
        _CACHE["nc"] = _build_program()
    return _CACHE["nc"]


def _make_in_maps(x, Wq_h, Wkv_h, Wout_h, bout_h, Wq_w, Wkv_w, Wout_w, bout_w,
                  conv_w, gamma, beta):
    import ml_dtypes
    f = np.float32
    bf = ml_dtypes.bfloat16
    shared = {
        "wq_w": np.ascontiguousarray(np.asarray(Wq_w, f).astype(bf)),
        "wk_w": np.ascontiguousarray(np.asarray(Wkv_w, f)[:, :C].astype(bf)),
        "wv_w": np.ascontiguousarray(np.asarray(Wkv_w, f)[:, C:].astype(bf)),
        "wo_w": np.ascontiguousarray(np.asarray(Wout_w, f).astype(bf)),
        "wq_h": np.ascontiguousarray(np.asarray(Wq_h, f).astype(bf)),
        "wk_h": np.ascontiguousarray(np.asarray(Wkv_h, f)[:, :C].astype(bf)),
        "wv_h": np.ascontiguousarray(np.asarray(Wkv_h, f)[:, C:].astype(bf)),
        "wo_h": np.ascontiguousarray(np.asarray(Wout_h, f).astype(bf)),
        "bsum": np.ascontiguousarray((np.asarray(bout_h, f) + np.asarray(bout_w, f)).reshape(C, 1)),
        "convA": np.ascontiguousarray(np.asarray(conv_w, f)[:C, :].astype(bf)),
        "convX": np.ascontiguousarray(np.asarray(conv_w, f)[C:, :].astype(bf)),
        "gamma2": np.ascontiguousarray(np.asarray(gamma, f).reshape(2, C).T),
        "beta2": np.ascontiguousarray(np.asarray(beta, f).reshape(2, C).T),
    }
    xb = np.asarray(x, f).astype(bf)
    return [{**shared, "x": np.ascontiguousarray(xb[b])} for b in range(B)]


def run(trace=False, **inputs):
    from concourse.bass_utils import run_bass_kernel_spmd

    nc = _get_program()
    in_maps = _make_in_maps(**inputs)
    res = run_bass_kernel_spmd(nc, in_maps, list(range(NCORES)), trace=trace)
    out = np.stack([res.results[b]["out"] for b in range(B)], axis=0)
    return out, res


def kernel(**inputs):
    out, _ = run(trace=False, **inputs)
    return out


# revision 34
# speedup vs baseline: 1.1926x; 1.1455x over previous
"""Trainium2 Bass kernel for BlockAxialDown (maxpool + axial attention + 1x1 conv + batchnorm).

Contract: kernel(**inputs) takes FULL unsharded inputs, returns FULL output.
Sharding: data-parallel over batch B=8 across 8 NeuronCores (1 image/core);
BatchNorm batch stats combined with a tiny (128,4) AllReduce; weights replicated.

v2: PE-based transposes (no Sync-engine DMA transposes), per-group batched
elementwise ops spread over Scalar/Vector/Pool engines, software-pipelined
issue order, maxpool interleaved with W-attention, single conv pass with the
relu'd conv output cached in SBUF (bf16) for the post-collective affine.
"""

import sys

import numpy as np

for _p in ("/opt/trn_rl_repo", "/root/.axon_site/_ro/trn_rl_repo"):
    if _p not in sys.path:
        sys.path.append(_p)

B, C, H, W = 8, 128, 256, 256
H2, W2 = 128, 128
E = 2 * C
NPOS = H2 * W2
NCORES = 8
BN_EPS = 1e-5
DH = C // 2
SCALE = DH ** -0.5

_CACHE = {}


def _build_program():
    import concourse.tile as tile
    from concourse import bacc, mybir, masks
    from concourse.alu_op_type import AluOpType
    from concourse.bass import broadcast_tensor_aps
    from contextlib import ExitStack

    F32 = mybir.dt.float32
    BF16 = mybir.dt.bfloat16
    AF = mybir.ActivationFunctionType
    AX = mybir.AxisListType
    P = 128

    nc = bacc.Bacc("TRN2", target_bir_lowering=False, debug=False, num_devices=NCORES)

    # ---- DRAM I/O ----
    x_d = nc.dram_tensor("x", [C, H, W], BF16, kind="ExternalInput").ap()
    wq_w_d = nc.dram_tensor("wq_w", [C, C], BF16, kind="ExternalInput").ap()
    wk_w_d = nc.dram_tensor("wk_w", [C, C], BF16, kind="ExternalInput").ap()
    wv_w_d = nc.dram_tensor("wv_w", [C, C], BF16, kind="ExternalInput").ap()
    wo_w_d = nc.dram_tensor("wo_w", [C, C], BF16, kind="ExternalInput").ap()
    wq_h_d = nc.dram_tensor("wq_h", [C, C], BF16, kind="ExternalInput").ap()
    wk_h_d = nc.dram_tensor("wk_h", [C, C], BF16, kind="ExternalInput").ap()
    wv_h_d = nc.dram_tensor("wv_h", [C, C], BF16, kind="ExternalInput").ap()
    wo_h_d = nc.dram_tensor("wo_h", [C, C], BF16, kind="ExternalInput").ap()
    bsum_d = nc.dram_tensor("bsum", [C, 1], F32, kind="ExternalInput").ap()
    convA_d = nc.dram_tensor("convA", [C, E], BF16, kind="ExternalInput").ap()
    convX_d = nc.dram_tensor("convX", [C, E], BF16, kind="ExternalInput").ap()
    gamma2_d = nc.dram_tensor("gamma2", [C, 2], F32, kind="ExternalInput").ap()
    beta2_d = nc.dram_tensor("beta2", [C, 2], F32, kind="ExternalInput").ap()
    out_d = nc.dram_tensor("out", [E, H2, W2], F32, kind="ExternalOutput").ap()
    stats_in_d = nc.dram_tensor("stats_in", [P, 4], F32).ap()
    stats_out_d = nc.dram_tensor("stats_out", [P, 4], F32, addr_space="Shared").ap()

    with tile.TileContext(nc) as tc, ExitStack() as ctx:
        const = ctx.enter_context(tc.tile_pool(name="const", bufs=1))
        cube = ctx.enter_context(tc.tile_pool(name="cube", bufs=1))
        stage = ctx.enter_context(tc.tile_pool(name="stage", bufs=3))
        work = ctx.enter_context(tc.tile_pool(name="work", bufs=2))
        stats = ctx.enter_context(tc.tile_pool(name="stats", bufs=1))
        psum = ctx.enter_context(tc.tile_pool(name="psum", bufs=1, space="PSUM"))

        # ---- constants ----
        def cload(name, ap_d, shape, dt):
            t = const.tile(shape, dt, name=name)
            nc.sync.dma_start(out=t[:], in_=ap_d)
            return t

        wq_w = cload("wq_w_t", wq_w_d, [C, C], BF16)
        wk_w = cload("wk_w_t", wk_w_d, [C, C], BF16)
        wv_w = cload("wv_w_t", wv_w_d, [C, C], BF16)
        wo_w = cload("wo_w_t", wo_w_d, [C, C], BF16)
        wq_h = cload("wq_h_t", wq_h_d, [C, C], BF16)
        wk_h = cload("wk_h_t", wk_h_d, [C, C], BF16)
        wv_h = cload("wv_h_t", wv_h_d, [C, C], BF16)
        wo_h = cload("wo_h_t", wo_h_d, [C, C], BF16)
        bsum = cload("bsum_t", bsum_d, [C, 1], F32)
        convA = cload("convA_t", convA_d, [C, E], BF16)
        convX = cload("convX_t", convX_d, [C, E], BF16)
        gamma2 = cload("gamma2_t", gamma2_d, [C, 2], F32)
        beta2 = cload("beta2_t", beta2_d, [C, 2], F32)
        ident = const.tile([P, P], BF16)
        masks.make_identity(nc, ident[:])

        xp = cube.tile([P, H2, W2], BF16)   # pooled input, channels on partitions
        acc = cube.tile([P, H2, W2], BF16)  # attention output accumulator
        ysb = [cube.tile([P, NPOS], BF16, name=f"ysb{i}") for i in range(2)]
        xp_f = xp[:].rearrange("c h w -> c (h w)")
        acc_f = acc[:].rearrange("c h w -> c (h w)")

        # ---- 2x2 maxpool of 8 input rows -> 4 pooled rows ----
        xv = x_d.rearrange("c (n r) w -> c n r w", r=8)

        def pool_iter(i):
            xin = stage.tile([P, 8, W], BF16, tag="xin")
            nc.sync.dma_start(out=xin[:], in_=xv[:, i])
            t = stage.tile([P, 8, W2], BF16, tag="wmax")
            xin4 = xin[:].rearrange("c r (w two) -> c r w two", two=2)
            nc.vector.tensor_max(t[:], xin4[:, :, :, 0], xin4[:, :, :, 1])
            t4 = t[:].rearrange("c (r2 two) w -> c r2 two w", two=2)
            nc.vector.tensor_max(xp[:, 4 * i:4 * i + 4, :], t4[:, :, 0, :], t4[:, :, 1, :])

        # ---- axial attention group (4 slices), 3-stage software pipeline ----
        def attn_A1(g, horiz):
            if horiz:
                wq, wk, wv = wq_h, wk_h, wv_h
                rhs_g = xp[:, :, 4 * g:4 * g + 4].rearrange("c h w -> c w h")
            else:
                wq, wk, wv = wq_w, wk_w, wv_w
                rhs_g = xp[:, 4 * g:4 * g + 4, :]
            # q/k/v into one fused 3-bank PSUM tile; qk evac on S, v on V
            qkv_ps = psum.tile([P, 1536], F32, tag="qkv", name="qkv_ps")
            nc.tensor.matmul(qkv_ps[:, 0:512], lhsT=wq[:], rhs=rhs_g, start=True, stop=True)
            nc.tensor.matmul(qkv_ps[:, 512:1024], lhsT=wk[:], rhs=rhs_g, start=True, stop=True)
            for s in range(4):
                nc.tensor.matmul(qkv_ps[:, 1024 + 128 * s:1152 + 128 * s],
                                 lhsT=rhs_g[:, s, :], rhs=wv[:], start=True, stop=True)
            qkv = work.tile([P, 1536], BF16, tag="qkv_sb", bufs=5)
            if horiz:
                nc.scalar.copy(qkv[:, 0:1024], qkv_ps[:, 0:1024])
                nc.vector.tensor_copy(qkv[:, 1024:1536], qkv_ps[:, 1024:1536])
            else:
                # W-phase: Vector is saturated by the pool maxes; evac all on S
                nc.scalar.copy(qkv[:], qkv_ps[:])
            return {"g": g, "horiz": horiz, "qkv": qkv}

        def attn_A2(st):
            qkv = st["qkv"]
            # dots for both heads in one 2-bank PSUM tile: cols 512*h + 128*s
            d_ps = psum.tile([P, 1024], F32, tag="d", name="d_ps")
            for s in range(4):
                for h in range(2):
                    hp = slice(64 * h, 64 * h + 64)
                    nc.tensor.matmul(d_ps[:, 512 * h + 128 * s:512 * h + 128 * s + 128],
                                     lhsT=qkv[hp, 128 * s:128 * s + 128],
                                     rhs=qkv[hp, 512 + 128 * s:640 + 128 * s],
                                     start=True, stop=True)
            e_sb = work.tile([P, 1024], BF16, tag="e", name="e_sb", bufs=3)
            nc.scalar.activation(e_sb[:], d_ps[:], AF.Exp, scale=SCALE)
            ev = e_sb[:].rearrange("c (hs j) -> c hs j", j=128)
            sums = work.tile([P, 8], F32, tag="sums", bufs=3)
            nc.vector.tensor_reduce(sums[:], ev, axis=AX.X, op=AluOpType.add)
            rcp = work.tile([P, 8], F32, tag="rcp", bufs=3)
            nc.vector.reciprocal(rcp[:], sums[:])
            rcp_v = rcp[:].rearrange("c (hs one) -> c hs one", one=1)
            _, rb = broadcast_tensor_aps(ev, rcp_v)
            nc.gpsimd.tensor_tensor(out=ev, in0=ev, in1=rb, op=AluOpType.mult)
            st["e_sb"] = e_sb
            return st

        # ---- stage B: transpose e, attend, project out, write acc ----
        def attn_B(st):
            g, horiz, e_sb, qkv = st["g"], st["horiz"], st["e_sb"], st["qkv"]
            wo = wo_h if horiz else wo_w
            et_ps = psum.tile([P, 1024], BF16, tag="et", name="et_ps")
            for s in range(4):
                for h in range(2):
                    o = 256 * s + 128 * h
                    nc.tensor.transpose(et_ps[:, o:o + 128],
                                        e_sb[:, 512 * h + 128 * s:512 * h + 128 * s + 128],
                                        ident[:])
            et = work.tile([P, 1024], BF16, tag="et_sb")
            nc.scalar.copy(et[:], et_ps[:])
            og_ps = psum.tile([P, 512], F32, tag="og", name="og_ps")
            for s in range(4):
                for h in range(2):
                    o = 256 * s + 128 * h
                    nc.tensor.matmul(og_ps[64 * h:64 * h + 64, 128 * s:128 * s + 128],
                                     lhsT=qkv[:, 1024 + 128 * s + 64 * h:1088 + 128 * s + 64 * h],
                                     rhs=et[:, o:o + 128], start=True, stop=True,
                                     tile_position=(0, 64 * h))
            og = work.tile([P, 512], BF16, tag="og_sb")
            nc.vector.tensor_copy(og[:], og_ps[:])
            yg_ps = psum.tile([P, 512], F32, tag="yg", name="yg_ps")
            nc.tensor.matmul(yg_ps[:], lhsT=wo[:], rhs=og[:], start=True, stop=True)
            if not horiz:
                # acc = yT_w + (bout_h + bout_w), contiguous write
                nc.scalar.activation(acc_f[:, 512 * g:512 * (g + 1)], yg_ps[:],
                                     AF.Identity, bias=bsum[:, 0:1], scale=1.0)
            else:
                # accumulate transposed: acc[:, h, w] += yg[:, (s=w, i=h)]
                acc_sl = acc[:, :, 4 * g:4 * g + 4]
                yg_r = yg_ps[:].rearrange("c (s i) -> c i s", s=4)
                nc.vector.tensor_add(acc_sl, acc_sl, yg_r)

        # ---- interleaved pool + attention, 3-stage software pipeline ----
        # lags: A1 at i, A2 at i-2, B at i-4; pool runs 3 steps ahead of A1 so
        # the projection matmuls never wait on fresh pool output. Within a step,
        # ready work (A2/B of older groups) issues before the new group's A1.
        seq = [(g, False) for g in range(H2 // 4)] + [(g, True) for g in range(W2 // 4)]
        n = len(seq)
        LAG_A2, LAG_B, POOL_AHEAD = 2, 4, 3
        for i in range(POOL_AHEAD):
            pool_iter(i)
        stash = {}
        for idx in range(n + LAG_B):
            j = idx - LAG_A2
            if 0 <= j < n:
                stash[j] = attn_A2(stash[j])
            j = idx - LAG_B
            if 0 <= j < n:
                attn_B(stash.pop(j))
            if idx + POOL_AHEAD < H2 // 4:
                pool_iter(idx + POOL_AHEAD)
            if idx < n:
                g, horiz = seq[idx]
                stash[idx] = attn_A1(g, horiz)

        # ---- relu over acc (split S/V) ----
        for j in range(4):
            sl = acc_f[:, 4096 * j:4096 * (j + 1)]
            if j % 2 == 0:
                nc.vector.tensor_scalar_max(sl, sl, 0.0)
            else:
                nc.scalar.activation(sl, sl, AF.Relu)

        # ---- conv pass (single): matmul + relu -> ysb (bf16), bn stats ----
        bnb = [stats.tile([P, 32, 6], F32, name=f"bnb{i}") for i in range(2)]
        for p in range(NPOS // 512):
            pos = slice(512 * p, 512 * (p + 1))
            for eh in range(2):
                ce = slice(128 * eh, 128 * eh + 128)
                yps = psum.tile([P, 512], F32, tag=("yg" if eh == 0 else "et"),
                                name=f"conv_ps{eh}")
                nc.tensor.matmul(yps[:], lhsT=convA[:, ce], rhs=acc_f[:, pos],
                                 start=True, stop=False)
                nc.tensor.matmul(yps[:], lhsT=convX[:, ce], rhs=xp_f[:, pos],
                                 start=False, stop=True)
                nc.scalar.activation(ysb[eh][:, pos], yps[:], AF.Relu)
                nc.vector.bn_stats(bnb[eh][:, p, :], ysb[eh][:, pos])

        # ---- global BN stats via AllReduce ----
        mv = stats.tile([P, 2, 2], F32)
        for eh in range(2):
            nc.vector.bn_aggr(mv[:, eh, :], bnb[eh][:])
        cc_in = stats.tile([P, 4], F32)
        for eh in range(2):
            # [mean, E[y^2]] per half; E[y^2] = var + mean^2
            nc.vector.tensor_copy(cc_in[:, 2 * eh:2 * eh + 1], mv[:, eh, 0:1])
            nc.vector.scalar_tensor_tensor(
                cc_in[:, 2 * eh + 1:2 * eh + 2],
                in0=mv[:, eh, 0:1], scalar=mv[:, eh, 0:1], in1=mv[:, eh, 1:2],
                op0=AluOpType.mult, op1=AluOpType.add)
        nc.sync.dma_start(out=stats_in_d, in_=cc_in[:])
        nc.gpsimd.collective_compute(
            "AllReduce", AluOpType.add,
            replica_groups=[list(range(NCORES))],
            ins=[stats_in_d], outs=[stats_out_d])
        gst = stats.tile([P, 4], F32)
        nc.sync.dma_start(out=gst[:], in_=stats_out_d)

        # ---- BN affine coefficients ----
        t0 = stats.tile([P, 4], F32)
        nc.vector.tensor_scalar_mul(t0[:], gst[:], 1.0 / NCORES)
        t0v = t0[:].rearrange("c (e two) -> c e two", two=2)
        m2 = stats.tile([P, 2], F32)
        veps = stats.tile([P, 2], F32)
        for eh in range(2):
            nc.vector.tensor_mul(m2[:, eh:eh + 1], t0v[:, eh, 0:1], t0v[:, eh, 0:1])
            nc.vector.scalar_tensor_tensor(
                veps[:, eh:eh + 1],
                in0=t0v[:, eh, 1:2], scalar=BN_EPS, in1=m2[:, eh:eh + 1],
                op0=AluOpType.add, op1=AluOpType.subtract)
        sd = stats.tile([P, 2], F32)
        nc.scalar.sqrt(sd[:], veps[:])
        rstd = stats.tile([P, 2], F32)
        nc.vector.reciprocal(rstd[:], sd[:])
        scl = stats.tile([P, 2], F32)
        nc.vector.tensor_mul(scl[:], gamma2[:], rstd[:])
        msc = stats.tile([P, 2], F32)
        means = stats.tile([P, 2], F32)
        nc.vector.tensor_copy(means[:, 0:1], t0v[:, 0, 0:1])
        nc.vector.tensor_copy(means[:, 1:2], t0v[:, 1, 0:1])
        nc.vector.tensor_mul(msc[:], means[:], scl[:])
        shift = stats.tile([P, 2], F32)
        nc.vector.tensor_sub(shift[:], beta2[:], msc[:])

        # ---- affine + store from cached ysb (Scalar: y*scl + shift) ----
        out_r = out_d.rearrange("(two c) h w -> two c (h w)", two=2)
        for p in range(NPOS // 512):
            pos = slice(512 * p, 512 * (p + 1))
            for eh in range(2):
                yo = work.tile([P, 512], F32, tag=f"yo{eh}")
                if eh == 0:
                    nc.scalar.activation(yo[:], ysb[0][:, pos], AF.Identity,
                                         scale=scl[:, 0:1], bias=shift[:, 0:1])
                else:
                    nc.vector.tensor_scalar(
                        yo[:], ysb[1][:, pos], scl[:, 1:2], shift[:, 1:2],
                        op0=AluOpType.mult, op1=AluOpType.add)
                nc.sync.dma_start(out=out_r[eh, :, pos], in_=yo[:])

    nc.finalize()
    return nc


def _get_program():
    if "nc" not in _CACHE:
        _CACHE["nc"] = _build_program()
    return _CACHE["nc"]


def _make_in_maps(x, Wq_h, Wkv_h, Wout_h, bout_h, Wq_w, Wkv_w, Wout_w, bout_w,
                  conv_w, gamma, beta):
    import ml_dtypes
    f = np.float32
    bf = ml_dtypes.bfloat16
    shared = {
        "wq_w": np.ascontiguousarray(np.asarray(Wq_w, f).astype(bf)),
        "wk_w": np.ascontiguousarray(np.asarray(Wkv_w, f)[:, :C].astype(bf)),
        "wv_w": np.ascontiguousarray(np.asarray(Wkv_w, f)[:, C:].astype(bf)),
        "wo_w": np.ascontiguousarray(np.asarray(Wout_w, f).astype(bf)),
        "wq_h": np.ascontiguousarray(np.asarray(Wq_h, f).astype(bf)),
        "wk_h": np.ascontiguousarray(np.asarray(Wkv_h, f)[:, :C].astype(bf)),
        "wv_h": np.ascontiguousarray(np.asarray(Wkv_h, f)[:, C:].astype(bf)),
        "wo_h": np.ascontiguousarray(np.asarray(Wout_h, f).astype(bf)),
        "bsum": np.ascontiguousarray((np.asarray(bout_h, f) + np.asarray(bout_w, f)).reshape(C, 1)),
        "convA": np.ascontiguousarray(np.asarray(conv_w, f)[:C, :].astype(bf)),
        "convX": np.ascontiguousarray(np.asarray(conv_w, f)[C:, :].astype(bf)),
        "gamma2": np.ascontiguousarray(np.asarray(gamma, f).reshape(2, C).T),
        "beta2": np.ascontiguousarray(np.asarray(beta, f).reshape(2, C).T),
    }
    xb = np.asarray(x, f).astype(bf)
    return [{**shared, "x": np.ascontiguousarray(xb[b])} for b in range(B)]


def run(trace=False, **inputs):
    from concourse.bass_utils import run_bass_kernel_spmd

    nc = _get_program()
    in_maps = _make_in_maps(**inputs)
    res = run_bass_kernel_spmd(nc, in_maps, list(range(NCORES)), trace=trace)
    out = np.stack([res.results[b]["out"] for b in range(B)], axis=0)
    return out, res


def kernel(**inputs):
    out, _ = run(trace=False, **inputs)
    return out
